# revision 9
# baseline (speedup 1.0000x reference)
"""PhaseAwareQuantization Trainium2 kernel (8-core SPMD, full-I/O).

Strategy (T-sharded): core i computes outputs for t in [512*i, 512*(i+1))
over all batches.
  - analytic-signal phase: the Hilbert transform is a circulant matmul
    (imag kernel of ifft(h)); even/odd parity halves the MACs. Computed on
    the PE as 64 accumulating matmuls against precomputed circulant slices.
  - atan2 via half-angle: phi = 2*atan(y / (sqrt(x^2+y^2) + x)); Arctan /
    Sqrt / Ln / Exp / Square on ACT, adds/muls on DVE.
  - features: single K=12 matmul (9 imu channels + cos + sin + ones) against
    a packed [12,65] weight whose 65th output row is the constant 1 (bias
    row for the VQ matmul); VQ scores: K=65 matmul against
    [codebook^T; -||c||^2/2] so that argmax(score) == argmin d2.
  - argmin over 512 codes per position: prefix-max scan + count(pm < max)
    = first-occurrence argmax; exact, 2 passes, split between the DVE and
    GPSIMD engines.
  - quantized = codebook[indices] is assembled host-side (pure gather).
"""
import numpy as np

B, C, T = 32, 9, 4096
NCORES = 8
TS = T // NCORES          # 512 t per core
TH = T // 2               # 2048
MH = TS // 2              # 256 outputs per parity per core
NSIG = B * C              # 288
NCODE, DIM, KV = 512, 64, 65
NBT = B * TS              # 16384 positions per core
NCH = NBT // 128          # 128 chunks of 128 positions
NKT = TH // 128           # 16 K tiles for hilbert matmul
NF = NBT // 512           # 32 feature chunks

# chunk j handled by GPSIMD iff GP_SEL[j]
GP_SEL = [(j % 8) < 5 for j in range(NCH)]

_CACHE = {}


def _build():
    import concourse.bass as bass
    import concourse.mybir as mybir
    from concourse import bacc
    from concourse.tile import TileContext
    from concourse.masks import make_identity

    f32 = mybir.dt.float32
    Alu = mybir.AluOpType
    Act = mybir.ActivationFunctionType

    nc = bacc.Bacc("TRN2", num_devices=NCORES)

    # ---- DRAM I/O ----
    xte = nc.dram_tensor("xte", [TH, NSIG], f32, kind="ExternalInput")
    xto = nc.dram_tensor("xto", [TH, NSIG], f32, kind="ExternalInput")
    he = nc.dram_tensor("he", [TH, MH], f32, kind="ExternalInput")
    ho = nc.dram_tensor("ho", [TH, MH], f32, kind="ExternalInput")
    xse = nc.dram_tensor("xse", [MH, NSIG], f32, kind="ExternalInput")
    xso = nc.dram_tensor("xso", [MH, NSIG], f32, kind="ExternalInput")
    xsl = nc.dram_tensor("xsl", [C + 1, NBT], f32, kind="ExternalInput")
    wcb = nc.dram_tensor("wcb", [12, NCODE], f32, kind="ExternalInput")
    idxd = nc.dram_tensor("idxd", [128, NCH], f32, kind="ExternalOutput")
    pout = nc.dram_tensor("pout", [B, TS], f32, kind="ExternalOutput")

    with TileContext(nc) as tc:
        with tc.tile_pool(name="persist", bufs=1) as pp:
            wcb_t = pp.tile([12, NCODE], f32, tag="wcb")
            nc.sync.dma_start(out=wcb_t, in_=wcb[:, :])
            ones_t = pp.tile([128, NCODE], f32, tag="ones")
            nc.vector.memset(ones_t, 1.0)
            ident = pp.tile([128, 128], f32, tag="ident")
            make_identity(nc, ident[:, :])
            phsb = pp.tile([B, TS], f32, tag="phsb")
            cssb = pp.tile([B, TS], f32, tag="cssb")
            snsb = pp.tile([B, TS], f32, tag="snsb")
            idxd_sb = pp.tile([128, NCH], f32, tag="idxd")
            nc.vector.memset(idxd_sb, 0.0)
            tiny = pp.tile([128, 1], f32, tag="tiny")
            nc.vector.memset(tiny, 1e-30)

            # ================= stage 1: hilbert + phases =================
            with tc.tile_pool(name="hilb", bufs=1) as hp, \
                 tc.tile_pool(name="ph", bufs=2) as ph, \
                 tc.tile_pool(name="hps", bufs=2, space="PSUM") as hps, \
                 tc.tile_pool(name="tps", bufs=2, space="PSUM") as tps:

                xte_t = [hp.tile([128, NSIG], f32, tag=f"xte{k}", name=f"xte{k}")
                         for k in range(NKT)]
                xto_t = [hp.tile([128, NSIG], f32, tag=f"xto{k}", name=f"xto{k}")
                         for k in range(NKT)]
                he_t = [hp.tile([128, MH], f32, tag=f"he{k}", name=f"het{k}")
                        for k in range(NKT)]
                ho_t = [hp.tile([128, MH], f32, tag=f"ho{k}", name=f"hot{k}")
                        for k in range(NKT)]
                for k in range(NKT):
                    s = slice(128 * k, 128 * (k + 1))
                    nc.sync.dma_start(out=xte_t[k], in_=xte[s, :])
                    nc.sync.dma_start(out=xto_t[k], in_=xto[s, :])
                    nc.sync.dma_start(out=he_t[k], in_=he[s, :])
                    nc.sync.dma_start(out=ho_t[k], in_=ho[s, :])
                xse_t = [hp.tile([128, NSIG], f32, tag=f"xse{m}", name=f"xset{m}")
                         for m in range(2)]
                xso_t = [hp.tile([128, NSIG], f32, tag=f"xso{m}", name=f"xsot{m}")
                        for m in range(2)]
                for m in range(2):
                    s = slice(128 * m, 128 * (m + 1))
                    nc.sync.dma_start(out=xse_t[m], in_=xse[s, :])
                    nc.sync.dma_start(out=xso_t[m], in_=xso[s, :])

                for ci, (par, mc) in enumerate([(0, 0), (0, 1), (1, 0), (1, 1)]):
                    hw_t = he_t if par == 0 else ho_t
                    rhs_t = xto_t if par == 0 else xte_t
                    xv_t = (xse_t if par == 0 else xso_t)[mc]
                    ps = hps.tile([128, NSIG], f32)
                    for k in range(NKT):
                        nc.tensor.matmul(
                            ps,
                            lhsT=hw_t[k][:, 128 * mc:128 * (mc + 1)],
                            rhs=rhs_t[k],
                            start=(k == 0), stop=(k == NKT - 1))
                    # atan2(y=ps, x=xv_t) via half angle; *2 and /9 at the end
                    x2 = ph.tile([128, NSIG], f32, tag="x2")
                    nc.scalar.activation(x2[:, :], xv_t[:, :], Act.Square)
                    y2 = ph.tile([128, NSIG], f32, tag="y2")
                    nc.scalar.activation(y2[:, :], ps[:, :], Act.Square)
                    r2 = ph.tile([128, NSIG], f32, tag="r2")
                    nc.vector.tensor_tensor(out=r2, in0=x2, in1=y2, op=Alu.add)
                    r = ph.tile([128, NSIG], f32, tag="r")
                    nc.scalar.activation(r[:, :], r2[:, :], Act.Sqrt)
                    d = ph.tile([128, NSIG], f32, tag="d")
                    nc.vector.tensor_tensor(out=d, in0=r, in1=xv_t[:, :],
                                            op=Alu.add)
                    d2 = ph.tile([128, NSIG], f32, tag="d2")
                    nc.scalar.activation(d2[:, :], d[:, :], Act.Square)
                    s2 = ph.tile([128, NSIG], f32, tag="s2")
                    nc.vector.tensor_tensor(out=s2, in0=d2, in1=y2, op=Alu.add)
                    rb = ph.tile([128, NSIG], f32, tag="rb")
                    nc.scalar.activation(rb[:, :], s2[:, :], Act.Sqrt)
                    e = ph.tile([128, NSIG], f32, tag="e")
                    nc.vector.tensor_tensor(out=e, in0=rb, in1=d, op=Alu.add)
                    ld = ph.tile([128, NSIG], f32, tag="ld")
                    nc.scalar.activation(ld[:, :], e[:, :], Act.Ln,
                                         bias=tiny[:, :])
                    invd = ph.tile([128, NSIG], f32, tag="invd")
                    nc.scalar.activation(invd[:, :], ld[:, :], Act.Exp,
                                         scale=-1.0)
                    q = ph.tile([128, NSIG], f32, tag="q")
                    nc.vector.tensor_tensor(out=q, in0=ps[:, :], in1=invd,
                                            op=Alu.mult)
                    a = ph.tile([128, NSIG], f32, tag="a")
                    nc.scalar.activation(a[:, :], q[:, :], Act.Arctan)
                    am = ph.tile([128, B], f32, tag="am")
                    nc.vector.tensor_reduce(
                        out=am, in_=a[:, :].rearrange("p (b c) -> p b c", c=C),
                        axis=mybir.AxisListType.X, op=Alu.add)
                    tp = tps.tile([B, 128], f32)
                    nc.tensor.transpose(tp[:, :], am[:, :], ident[:, :])
                    # interleave parity into natural t order; *2, /9
                    s0 = par + 256 * mc
                    dst = phsb[:, s0:min(s0 + 256, TS):2]
                    nc.scalar.activation(dst, tp[:, :], Act.Copy,
                                         scale=4.0 / 9.0)

            # ---- phases out; cos/sin ----
            nc.sync.dma_start(out=pout[:, :], in_=phsb[:, :])
            nc.scalar.activation(snsb[:, :], phsb[:, :], Act.Sin)
            shf = pp.tile([B, TS], f32, tag="shf")
            nc.scalar.activation(shf[:, :], phsb[:, :], Act.Sin, scale=0.5)
            nc.scalar.activation(shf[:, :], shf[:, :], Act.Square)
            nc.scalar.activation(cssb[:, :], shf[:, :], Act.Copy,
                                 bias=1.0, scale=-2.0)

            # ================= stage 2: VQ =================
            with tc.tile_pool(name="vt", bufs=1) as vtp, \
                 tc.tile_pool(name="pm", bufs=4) as pmp, \
                 tc.tile_pool(name="scrap", bufs=2) as scp, \
                 tc.tile_pool(name="vqp", bufs=4, space="PSUM") as vqp:

                v_t = vtp.tile([12, NBT], f32, tag="v")
                nc.sync.dma_start(out=v_t[0:9, :], in_=xsl[0:9, :])
                nc.sync.dma_start(out=v_t[11:12, :], in_=xsl[9:10, :])
                nc.sync.dma_start(out=v_t[9:10, :], in_=cssb[:, :])
                nc.sync.dma_start(out=v_t[10:11, :], in_=snsb[:, :])

                for j in range(NCH):
                    vp = vqp.tile([128, NCODE], f32)
                    nc.tensor.matmul(
                        vp, lhsT=v_t[:, 128 * j:128 * (j + 1)],
                        rhs=wcb_t[:, :], start=True, stop=True)
                    pm = pmp.tile([128, NCODE], f32, tag="pm")
                    nc.vector.tensor_tensor_scan(
                        out=pm, data0=vp, data1=ones_t,
                        initial=-1e30, op0=Alu.max, op1=Alu.bypass)
                    sd = scp.tile([128, NCODE], f32, tag="scrapd")
                    nc.scalar.activation(
                        sd[:, :], pm[:, :], Act.Sign, scale=-1.0,
                        bias=pm[:, NCODE - 1:NCODE],
                        accum_out=idxd_sb[:, j:j + 1])

                nc.sync.dma_start(out=idxd[:, :], in_=idxd_sb)

    nc.compile()
    return nc


def _host_prep(imu_signal, W_mag, b_mag, W_phase, b_phase, codebook):
    f = np.float32
    x = np.ascontiguousarray(imu_signal, dtype=f)
    # hilbert circulant kernel (imag part of ifft of the analytic multiplier)
    h = np.zeros(T, dtype=np.float64)
    h[0] = 1.0
    h[1:T // 2] = 2.0
    h[T // 2] = 1.0
    g = np.fft.ifft(h).imag.astype(f)  # g[d], zero at even d

    xT = x.reshape(NSIG, T).T  # [T, sig]
    xte = np.ascontiguousarray(xT[0::2])  # [TH, NSIG]
    xto = np.ascontiguousarray(xT[1::2])

    j_ = np.arange(TH)[:, None]
    m_ = np.arange(MH)[None, :]

    # packed feature weight; output row 64 is the constant 1 (VQ bias row)
    M = np.zeros((KV, 12), dtype=f)
    M[0:32, 0:9] = W_mag
    M[0:32, 11] = b_mag
    M[32:64, 0:7] = W_phase[:, 0:7]
    M[32:64, 9] = W_phase[:, 7]
    M[32:64, 10] = W_phase[:, 8]
    M[32:64, 11] = b_phase
    M[64, 11] = 1.0

    cbf = np.asarray(codebook, dtype=np.float64)
    cba = np.concatenate(
        [cbf.T, -0.5 * (cbf ** 2).sum(1)[None, :]], axis=0)
    wcb = np.ascontiguousarray(
        (M.astype(np.float64).T @ cba).astype(f))

    in_maps = []
    for i in range(NCORES):
        t0 = TS * i
        he_i = np.ascontiguousarray(g[(2 * (MH * i + m_) - 2 * j_ - 1) % T])
        ho_i = np.ascontiguousarray(g[(2 * (MH * i + m_) + 1 - 2 * j_) % T])
        xsl_i = np.concatenate(
            [x[:, :, t0:t0 + TS].transpose(1, 0, 2).reshape(C, NBT),
             np.ones((1, NBT), dtype=f)], axis=0)
        in_maps.append({
            "xte": xte, "xto": xto, "he": he_i, "ho": ho_i,
            "xse": np.ascontiguousarray(xte[MH * i:MH * (i + 1)]),
            "xso": np.ascontiguousarray(xto[MH * i:MH * (i + 1)]),
            "xsl": np.ascontiguousarray(xsl_i), "wcb": wcb,
        })
    return in_maps


def kernel(imu_signal, W_mag, b_mag, W_phase, b_phase, codebook):
    from concourse.bass_utils import run_bass_kernel_spmd

    if "nc" not in _CACHE:
        _CACHE["nc"] = _build()
    nc = _CACHE["nc"]

    in_maps = _host_prep(imu_signal, W_mag, b_mag, W_phase, b_phase, codebook)
    res = run_bass_kernel_spmd(nc, in_maps, core_ids=list(range(NCORES)))

    indices = np.empty((B, T), dtype=np.int32)
    phases = np.empty((B, T), dtype=np.float32)
    for i in range(NCORES):
        r = res.results[i]
        idx_bt = r["idxd"].T.reshape(NBT)  # bt = 128*chunk + p
        indices[:, TS * i:TS * (i + 1)] = \
            idx_bt.reshape(B, TS).astype(np.int32)
        phases[:, TS * i:TS * (i + 1)] = r["pout"]

    quantized = np.asarray(codebook, dtype=np.float32)[indices]
    return quantized, indices, phases
